# revision 1
# baseline (speedup 1.0000x reference)
"""v3: engine-balanced CVLoss kernel.

Per chunk (W=2000, per-chunk-local positions, unchained scans):
  ACT : xb=cast(x) accum->k ; Square(D) accum->s2 ; Tanh(1000*M) accum->z'
  DVE : v = iota*xb ; M = scan-max(v) ; D = M - Msh ; l = M[:,-1]
  DMA : chunk load ; Msh[:,1:] <- M[:,:-1] (aligned shifted copy)
  GPS : memset Msh[:,0:1]
Host: f_c = 2*(W - z'_c) + 1   (tanh(1000*M): 0 -> 0.5 via... see note)

Note on z': tanh(1000*M) = 0 at M=0? tanh(0)=0. M>=1 -> 1.0 exactly in f32.
So z' = count of M>0 directly (tanh(0)=0, not 0.5!) -> f_c = W - z' + 1.
acc layout: (P, 4*nch) = [k_c | s2_c | z_c | l_c].
"""

import numpy as np

B, T, N = 16, 2000, 512
L = B * T
NCORES = 8
NPC = N // NCORES
HALVES = 2
P = NPC * HALVES
F = L // HALVES
W = 2000
NCH = F // W

_BUILD_CACHE = {}


def build_bass(F_=F, W_=W, P_=P):
    import concourse.bass as bass
    from concourse import bacc
    import concourse.mybir as mybir
    from concourse import tile

    nch = F_ // W_
    Alu = mybir.AluOpType
    AF = mybir.ActivationFunctionType
    f32 = mybir.dt.float32
    i16 = mybir.dt.int16
    bf16 = mybir.dt.bfloat16

    nc = bacc.Bacc(trn_type="TRN2")
    x = nc.dram_tensor("x", (P_, F_), f32, kind="ExternalInput")
    acc = nc.dram_tensor("acc", (P_, 4 * nch), f32, kind="ExternalOutput")

    with tile.TileContext(nc) as tc:
        with tc.tile_pool(name="persist", bufs=1) as pp, \
             tc.tile_pool(name="work", bufs=4) as wp:
            iota = pp.tile([P_, W_], i16)
            nc.gpsimd.iota(iota[:], pattern=[[1, W_]], base=1, channel_multiplier=0)
            accs = pp.tile([P_, 4 * nch], f32)

            def load_and_cast(c):
                """DMA chunk c and cast it (ACT, accumulating k)."""
                lo = c * W_
                xc = wp.tile([P_, W_], f32, tag="xc", name=f"xc{c}")
                nc.sync.dma_start(out=xc[:], in_=x[:, lo:lo + W_])
                xb = wp.tile([P_, W_], i16, tag="xb", name=f"xb{c}")
                nc.scalar.activation(
                    out=xb[:], in_=xc[:], func=AF.Copy,
                    accum_out=accs[:, c:c + 1])
                return xb

            def compute(c, xb):
                """Scan/diff/reduce for chunk c (issued one chunk behind the
                cast so ACT's in-order queue never blocks the next cast)."""
                # v = iota * xb   (chunk-local 1-based positions)
                v = wp.tile([P_, W_], i16, tag="v", name=f"v{c}")
                nc.vector.tensor_tensor(
                    out=v[:], in0=iota[:], in1=xb[:], op=Alu.mult)
                # M = prefix max (last spike so far; 0 if none)
                M = wp.tile([P_, W_], i16, tag="M", name=f"M{c}")
                nc.vector.tensor_tensor_scan(
                    out=M[:], data0=v[:], data1=v[:], initial=0.0,
                    op0=Alu.max, op1=Alu.bypass)
                # D = diff of M (ISI at spikes incl. phantom-first, 0 elsewhere)
                D = wp.tile([P_, W_], i16, tag="D", name=f"D{c}")
                nc.vector.tensor_scalar(
                    out=D[:, 0:1], in0=M[:, 0:1], scalar1=0.0, scalar2=None,
                    op0=Alu.add)
                nc.vector.tensor_tensor(
                    out=D[:, 1:], in0=M[:, 1:W_], in1=M[:, 0:W_ - 1],
                    op=Alu.subtract)
                # s2 = sum D^2 on ACT
                dsq = wp.tile([P_, W_], bf16, tag="dsq", name=f"dsq{c}")
                nc.scalar.activation(
                    out=dsq[:], in_=D[:], func=AF.Square,
                    accum_out=accs[:, nch + c:nch + c + 1])
                # z = count of M>0 via tanh(1000*M) on ACT
                nz = wp.tile([P_, W_], bf16, tag="nz", name=f"nz{c}")
                nc.scalar.activation(
                    out=nz[:], in_=M[:], func=AF.Tanh, scale=1000.0,
                    accum_out=accs[:, 2 * nch + c:2 * nch + c + 1])
                # l_c = M[:, -1]  (i16 -> f32 via DVE ts copy)
                nc.vector.tensor_scalar(
                    out=accs[:, 3 * nch + c:3 * nch + c + 1],
                    in0=M[:, W_ - 1:W_], scalar1=0.0, scalar2=None,
                    op0=Alu.add)

            pending = None
            for c in range(nch):
                xb = load_and_cast(c)
                if pending is not None:
                    compute(*pending)
                pending = (c, xb)
            compute(*pending)

            nc.sync.dma_start(out=acc[:], in_=accs[:])
    nc.finalize()
    return nc


def get_bass():
    key = (F, W, P)
    if key not in _BUILD_CACHE:
        _BUILD_CACHE[key] = build_bass()
    return _BUILD_CACHE[key]


def shard_input(output_spikes):
    x = np.asarray(output_spikes, dtype=np.float32)
    maps = []
    for c in range(NCORES):
        xc = x[:, :, c * NPC:(c + 1) * NPC]
        xt = np.ascontiguousarray(np.transpose(xc, (2, 0, 1))).reshape(NPC, L)
        maps.append({"x": xt.reshape(P, F)})
    return maps


def finish_host(acc_list, target_cv, F_=F, W_=W, nch=NCH):
    """Merge per-(row, chunk) stats into the scalar loss (float64)."""
    target = np.asarray(target_cv, dtype=np.float64)
    sq_sum = 0.0
    n_valid = 0
    for ci, acc in enumerate(acc_list):
        a = np.asarray(acc, dtype=np.float64)
        P_ = a.shape[0]
        k_c = a[:, 0:nch]
        s2_c = a[:, nch:2 * nch]
        z_c = np.rint(a[:, 2 * nch:3 * nch])
        l_c = a[:, 3 * nch:4 * nch]
        f_c = W_ - z_c + 1.0
        n_neu = P_ // 2
        for n in range(n_neu):
            kt = 0.0
            s2 = 0.0
            gf = gl = None
            for h in range(2):
                p = n * 2 + h
                for s in range(nch):
                    ks = k_c[p, s]
                    if ks < 1:
                        continue
                    off = h * F_ + s * W_
                    s2r = s2_c[p, s] - f_c[p, s] ** 2
                    fg = off + f_c[p, s]
                    lg = off + l_c[p, s]
                    if gf is None:
                        gf = fg
                    else:
                        gap = fg - gl
                        s2 += gap * gap
                    s2 += s2r
                    gl = lg
                    kt += ks
            if kt < 3:
                continue
            s1 = gl - gf
            mean = s1 / (kt - 1.0)
            var = (s2 - s1 * s1 / (kt - 1.0)) / (kt - 2.0)
            std = np.sqrt(var) if var > 0 else 0.0
            if mean <= 0:
                continue
            cv = std / max(mean, 1e-12)
            d = cv - target[ci * NPC + n]
            sq_sum += d * d
            n_valid += 1
    return np.float32(sq_sum / max(n_valid, 1))



def ensure_ntff_hook(so_path="/opt/axon/libaxon_pjrt.so"):
    """Shim antenv.axon_hooks (absent in this image) so trace=True works.

    Mirrors trn_boot._ntff_profile_via_ctypes: drives NRT profiling via the
    axon PJRT .so's C ABI. Safe no-op if anything is missing.
    """
    import sys
    try:
        import antenv.axon_hooks  # noqa: F401
        return
    except ImportError:
        pass
    try:
        import ctypes
        import contextlib
        import types
        import os

        if not os.path.exists(so_path):
            return
        lib = ctypes.CDLL(so_path)
        if not hasattr(lib, "axon_start_nrt_profile"):
            return
        lib.axon_start_nrt_profile.argtypes = [
            ctypes.POINTER(ctypes.c_int64), ctypes.c_size_t]
        lib.axon_start_nrt_profile.restype = ctypes.c_int64
        lib.axon_stop_nrt_profile.argtypes = [ctypes.c_char_p]
        lib.axon_stop_nrt_profile.restype = ctypes.c_int64

        @contextlib.contextmanager
        def _hook(output_dir, device_ids):
            import jax
            jax.devices()
            if device_ids:
                ids = (ctypes.c_int64 * len(device_ids))(*device_ids)
                rc = lib.axon_start_nrt_profile(ids, len(device_ids))
            else:
                rc = lib.axon_start_nrt_profile(None, 0)
            if rc != 0:
                raise RuntimeError(f"axon_start_nrt_profile rc={rc}")
            try:
                yield
            finally:
                n = lib.axon_stop_nrt_profile(str(output_dir).encode())
                print(f"profile: {n} file(s) written to {output_dir}",
                      file=sys.stderr)

        mod = types.ModuleType("antenv.axon_hooks")
        mod.get_axon_ntff_profile_hook = lambda: _hook
        mod.set_axon_ntff_profile_hook = lambda h: None
        import antenv
        sys.modules["antenv.axon_hooks"] = mod
        antenv.axon_hooks = mod
    except Exception:
        pass



def kernel(output_spikes, target_cv):
    from concourse.bass_utils import run_bass_kernel_spmd

    ensure_ntff_hook()
    nc = get_bass()
    in_maps = shard_input(output_spikes)
    res = run_bass_kernel_spmd(nc, in_maps, core_ids=list(range(NCORES)))
    acc_list = [res.results[c]["acc"] for c in range(NCORES)]
    return finish_host(acc_list, target_cv)

